# revision 3
# baseline (speedup 1.0000x reference)
"""Attn_LSTM Trainium2 kernel — 8-core data-parallel Bass/Tile implementation.

Model (per reference): 1-layer LSTM encoder over L=96 steps, then T=24
attention-decoder steps. B=4096 sharded 512/core across 8 NeuronCores;
weights replicated.

Key device-side design points:
  * All recurrent state is kept transposed ([H, B]) so the PE consumes h
    directly as lhsT with no per-step transposes on the recurrent path.
  * Attention context uses a Horner-form affine scan (tensor_tensor_scan):
      S_l = r_l * S_{l-1} + enc_l,  r_l = e_{l-1}/e_l = exp(z_{l-1}-z_l)
    so softmax-weight-and-reduce is ONE DVE pass over enc. The z-differences
    come straight out of the PE by using column-differenced attention weights,
    and a -1e30 in the difference-bias column 0 makes exp()=0 there, which
    resets the scan at every (h-row, chunk) boundary for free. A ones-row
    appended to enc yields the softmax denominator from the same scan.
  * The local walrus build accepts at most ONE semaphore wait per
    instruction; legalize_waits() splits extra waits onto same-engine NoOps.
"""

import numpy as np

import concourse.bass as bass
import concourse.tile as tile
from concourse import mybir
from concourse.masks import make_identity
from concourse.bass_utils import run_bass_kernel_spmd

H = 64
C = 8
L = 96
T = 24
B = 4096
NCORES = 8
BS = B // NCORES          # 512 batch per core
NCH = BS // 128           # 4 partition chunks per core

F32 = mybir.dt.float32
AF = mybir.ActivationFunctionType
ALU = mybir.AluOpType

NEG_BIG = -1.0e30


def _legalize_waits(nc):
    """This walrus build rejects >1 sem wait per instruction; split extras
    onto same-engine NoOps placed immediately before."""
    cnt = 0
    for bb in nc.main_func.blocks:
        new = []
        for inst in bb.instructions:
            si = inst.sync_info
            if si is not None and len(si.on_wait) > 1:
                waits = list(si.on_wait)
                for w in waits[:-1]:
                    nop = mybir.InstNoOp(name=f"wsplit-{cnt}", ins=[], outs=[])
                    cnt += 1
                    nop.engine = inst.engine
                    nop.sync_info = mybir.SyncInfo(on_wait=[w], on_update=[])
                    new.append(nop)
                inst.sync_info = mybir.SyncInfo(
                    on_wait=[waits[-1]], on_update=list(si.on_update))
            new.append(inst)
        bb.instructions = new
    return cnt


def _tts_raw(nc, out, data0, data1, initial, op0, op1):
    """tensor_tensor_scan without the 2D-shape assert: the recurrence chains
    across free dims, which we exploit (r=0 at row starts resets the state)."""
    eng = nc.vector
    return eng.add_instruction(
        mybir.InstTensorScalarPtr(
            name=nc.get_next_instruction_name(),
            is_tensor_tensor_scan=True,
            is_scalar_tensor_tensor=True,
            op0=op0,
            op1=op1,
            ins=[
                eng.lower_ap(data0),
                eng.lower_ap_or_imm(initial),
                eng.lower_ap(data1),
            ],
            outs=[eng.lower_ap(out)],
        )
    )


def _build_program():
    nc = bass.Bass("TRN2", target_bir_lowering=False, debug=False,
                   num_devices=NCORES)

    ext_in = {}

    def din(name, shape):
        ext_in[name] = shape
        return nc.dram_tensor(name, list(shape), F32, kind="ExternalInput").ap()

    xT = din("xT", (L, C, BS))                 # normalized, transposed input
    enc_wih = din("enc_wih", (C, 4, H))        # gate order (i, f, o, g)
    enc_whh = din("enc_whh", (H, 4, H))
    enc_b = din("enc_b", (H, 4))
    emb_wT = din("emb_wT", (C, H))
    emb_b = din("emb_b", (H, 1))
    attn_we = din("attn_we", (H, L))           # DIFFERENCED emb-part weights
    attn_wh = din("attn_wh", (H + 1, L))       # rows 0-63 diff Wh, row 64 diff bias (+ -1e30 col0)
    wie = din("wie", (H, 4, H))                # dec_Wih @ comb_W1, packed
    wic = din("wic", (H, 4, H))                # dec_Wih @ comb_W2, packed
    dec_whh = din("dec_whh", (H, 4, H))
    dec_b = din("dec_b", (H, 4))
    out_wT = din("out_wT", (H, C))
    out_b = din("out_b", (C, 1))

    preds = nc.dram_tensor("preds", [T, C, BS], F32, kind="ExternalOutput").ap()

    with tile.TileContext(nc) as tc:
        with (
            tc.tile_pool(name="state", bufs=1) as st,
            tc.tile_pool(name="xin", bufs=4) as xin,
            tc.tile_pool(name="scan", bufs=2) as sc,
            tc.tile_pool(name="gps", bufs=1, space="PSUM") as gps,
            tc.tile_pool(name="tps", bufs=1, space="PSUM") as tps,
        ):
            # ---------- persistent tiles ----------
            ident = st.tile([128, 128], F32)
            make_identity(nc, ident[:])

            w_enc_wih = st.tile([C, 4, H], F32)
            w_enc_whh = st.tile([H, 4, H], F32)
            w_enc_b = st.tile([H, 4], F32)
            w_emb_wT = st.tile([C, H], F32)
            w_emb_b = st.tile([H, 1], F32)
            w_attn_we = st.tile([H, L], F32)
            w_attn_wh = st.tile([H + 1, L], F32)
            w_wie = st.tile([H, 4, H], F32)
            w_wic = st.tile([H, 4, H], F32)
            w_dec_whh = st.tile([H, 4, H], F32)
            w_dec_b = st.tile([H, 4], F32)
            w_out_wT = st.tile([H, C], F32)
            w_out_b = st.tile([C, 1], F32)
            for tl, ap in ((w_enc_wih, enc_wih), (w_enc_whh, enc_whh),
                           (w_enc_b, enc_b), (w_emb_wT, emb_wT),
                           (w_emb_b, emb_b), (w_attn_we, attn_we),
                           (w_attn_wh, attn_wh), (w_wie, wie), (w_wic, wic),
                           (w_dec_whh, dec_whh), (w_dec_b, dec_b),
                           (w_out_wT, out_wT), (w_out_b, out_b)):
                nc.gpsimd.dma_start(tl[:], ap[:])

            # recurrent state, transposed; h row 64 is a ones-row feeding the
            # (differenced) attention bias
            h_T = st.tile([H + 1, BS], F32)
            c_T = st.tile([H, BS], F32)
            nc.vector.memset(h_T[:], 0.0)
            nc.vector.memset(c_T[:], 0.0)
            nc.vector.memset(h_T[H : H + 1, :], 1.0)

            # encoder outputs, per 128-batch chunk: [b, h(+ones), l]
            enc_sb = [st.tile([128, H + 1, L], F32, tag=f"enc{ci}", name=f"enc{ci}")
                      for ci in range(NCH)]
            for ci in range(NCH):
                nc.vector.memset(enc_sb[ci][:, H, :], 1.0)

            # gate tiles (SBUF) + cell temps
            g_sb = [st.tile([H, BS], F32, tag=f"g{gi}", name=f"g{gi}") for gi in range(4)]
            t1 = st.tile([H, BS], F32)
            t2 = st.tile([H, BS], F32)
            tc_sb = st.tile([H, BS], F32)

            # decoder tiles
            emb_sb = st.tile([H, BS], F32)
            r_sb = st.tile([128, NCH, L], F32)
            rec_sb = st.tile([128, NCH], F32)
            ctx_ch = st.tile([128, NCH, H], F32)
            ctx_sb = st.tile([H, BS], F32)
            inp_sb = st.tile([C, BS], F32)

            # PSUM
            gate_ps = [gps.tile([H, BS], F32, tag=f"gp{gi}", name=f"gp{gi}") for gi in range(4)]
            tp_ps_pool = tps  # [128, NCH, H] tiles for encoder transposes

            ACTF = (AF.Sigmoid, AF.Sigmoid, AF.Sigmoid, AF.Tanh)

            def lstm_cell(bias_tile):
                """gates (psum) -> h_T/c_T update. Gate order (i, f, o, g)."""
                for gi in range(4):
                    nc.scalar.activation(g_sb[gi][:], gate_ps[gi][:], ACTF[gi],
                                         bias=bias_tile[:, gi : gi + 1])
                nc.vector.tensor_mul(t1[:], g_sb[0][:], g_sb[3][:])   # i*tanh(g)
                nc.vector.tensor_mul(t2[:], g_sb[1][:], c_T[:])       # f*c
                nc.vector.tensor_add(c_T[:], t1[:], t2[:])
                nc.scalar.activation(tc_sb[:], c_T[:], AF.Tanh)
                nc.vector.tensor_mul(h_T[0:H, :], g_sb[2][:], tc_sb[:])

            # ------------------ encoder ------------------
            for l in range(L):
                x_t = xin.tile([C, BS], F32, tag="x")
                nc.gpsimd.dma_start(x_t[:], xT[l])
                for gi in range(4):
                    nc.tensor.matmul(gate_ps[gi][:], w_enc_wih[:, gi, :],
                                     x_t[:], start=True, stop=False)
                    nc.tensor.matmul(gate_ps[gi][:], w_enc_whh[:, gi, :],
                                     h_T[0:H, :], start=False, stop=True)
                lstm_cell(w_enc_b)
                # store h (transposed back to [b, h]) into enc_sb[:, :, l]
                tp = tp_ps_pool.tile([128, NCH, H], F32, tag="tp")
                for ci in range(NCH):
                    nc.tensor.transpose(tp[:, ci, :],
                                        h_T[0:H, 128 * ci : 128 * (ci + 1)],
                                        ident[0:H, 0:H])
                    nc.scalar.copy(enc_sb[ci][:, 0:H, l], tp[:, ci, :])

            # ------------------ decoder ------------------
            # initial decoder input = last normalized x = xT[L-1]
            nc.gpsimd.dma_start(inp_sb[:], xT[L - 1])

            S_tiles = [sc.tile([128, H + 1, L], F32, tag="S", name=f"S{si}")
                       for si in range(2)]

            for t in range(T):
                # embedding
                emb_ps = tps.tile([H, BS], F32, tag="sm")
                nc.tensor.matmul(emb_ps[:], w_emb_wT[:], inp_sb[:],
                                 start=True, stop=True)
                nc.scalar.activation(emb_sb[:], emb_ps[:], AF.Relu,
                                     bias=w_emb_b[:, 0:1])

                # differenced attention scores -> exp -> scan ratios
                zd_ps = tps.tile([128, NCH, L], F32, tag="zd")
                for ci in range(NCH):
                    sl = slice(128 * ci, 128 * (ci + 1))
                    nc.tensor.matmul(zd_ps[:, ci, :], emb_sb[:, sl],
                                     w_attn_we[:], start=True, stop=False)
                    nc.tensor.matmul(zd_ps[:, ci, :], h_T[:, sl],
                                     w_attn_wh[:], start=False, stop=True)
                nc.scalar.activation(r_sb[:], zd_ps[:], AF.Exp)

                # Horner scan per chunk: S = r*S + enc  (flat over (h, l))
                ctxT_ps = tps.tile([H, BS], F32, tag="ctxT")
                for ci in range(NCH):
                    S = S_tiles[ci % 2]
                    rb = r_sb[:, ci, :].unsqueeze(1).broadcast_to((128, H + 1, L))
                    _tts_raw(nc, S[:], rb, enc_sb[ci][:], 0.0,
                             op0=ALU.mult, op1=ALU.add)
                    nc.vector.reciprocal(rec_sb[:, ci : ci + 1],
                                         S[:, H, L - 1 : L])
                    nc.vector.tensor_scalar(
                        out=ctx_ch[:, ci, :], in0=S[:, 0:H, L - 1],
                        scalar1=rec_sb[:, ci : ci + 1], scalar2=None,
                        op0=ALU.mult)
                    nc.tensor.transpose(ctxT_ps[:, 128 * ci : 128 * (ci + 1)],
                                        ctx_ch[:, ci, :], ident[:])
                nc.scalar.copy(ctx_sb[:], ctxT_ps[:])

                # decoder LSTM cell (comb layer folded into gate weights)
                for gi in range(4):
                    nc.tensor.matmul(gate_ps[gi][:], w_wie[:, gi, :],
                                     emb_sb[:], start=True, stop=False)
                    nc.tensor.matmul(gate_ps[gi][:], w_wic[:, gi, :],
                                     ctx_sb[:], start=False, stop=False)
                    nc.tensor.matmul(gate_ps[gi][:], w_dec_whh[:, gi, :],
                                     h_T[0:H, :], start=False, stop=True)
                lstm_cell(w_dec_b)

                # prediction -> next input + output store
                pred_ps = tps.tile([C, BS], F32, tag="sm")
                nc.tensor.matmul(pred_ps[:], w_out_wT[:], h_T[0:H, :],
                                 start=True, stop=True)
                nc.scalar.activation(inp_sb[:], pred_ps[:], AF.Identity,
                                     bias=w_out_b[:, 0:1])
                nc.gpsimd.dma_start(preds[t], inp_sb[:])

    _legalize_waits(nc)
    return nc


_NC_CACHE = []


def _get_nc():
    if not _NC_CACHE:
        _NC_CACHE.append(_build_program())
    return _NC_CACHE[0]


def _prep_weights(i):
    """Host-side packing. Gate order (i, f, o, g); PyTorch order is i,f,g,o."""
    idx = np.r_[0:64, 64:128, 192:256, 128:192]

    def pack(w):                       # [256, K] -> [K, 4, 64]
        return np.ascontiguousarray(
            w[idx].reshape(4, 64, -1).transpose(2, 0, 1).astype(np.float32))

    enc_wih = pack(i["enc_Wih"])
    enc_whh = pack(i["enc_Whh"])
    enc_b = np.ascontiguousarray(
        (i["enc_bih"] + i["enc_bhh"])[idx].reshape(4, 64).T.astype(np.float32))

    emb_wT = np.ascontiguousarray(i["emb_W"].T.astype(np.float32))
    emb_b = i["emb_b"].astype(np.float32).reshape(H, 1)

    # differenced attention weights: zd[:, l] = z[:, l-1] - z[:, l]
    we_T = i["attn_W"][:, :H].T.astype(np.float32)       # [64, 96]
    wh_T = i["attn_W"][:, H:].T.astype(np.float32)       # [64, 96]
    ab = i["attn_b"].astype(np.float32)                  # [96]
    we_d = np.zeros_like(we_T)
    we_d[:, 1:] = we_T[:, :-1] - we_T[:, 1:]
    wh_d = np.zeros((H + 1, L), np.float32)
    wh_d[:H, 1:] = wh_T[:, :-1] - wh_T[:, 1:]
    wh_d[H, 0] = NEG_BIG
    wh_d[H, 1:] = ab[:-1] - ab[1:]

    comb_W1 = i["comb_W"][:, :H].astype(np.float32)
    comb_W2 = i["comb_W"][:, H:].astype(np.float32)
    dec_Wih = i["dec_Wih"].astype(np.float32)
    wie = pack(dec_Wih @ comb_W1)
    wic = pack(dec_Wih @ comb_W2)
    dec_whh = pack(i["dec_Whh"])
    dec_b_full = (i["dec_bih"] + i["dec_bhh"] + dec_Wih @ i["comb_b"])
    dec_b = np.ascontiguousarray(
        dec_b_full[idx].reshape(4, 64).T.astype(np.float32))

    out_wT = np.ascontiguousarray(i["out_W"].T.astype(np.float32))
    out_b = i["out_b"].astype(np.float32).reshape(C, 1)

    return dict(enc_wih=enc_wih, enc_whh=enc_whh, enc_b=enc_b,
                emb_wT=emb_wT, emb_b=emb_b, attn_we=np.ascontiguousarray(we_d),
                attn_wh=np.ascontiguousarray(wh_d), wie=wie, wic=wic,
                dec_whh=dec_whh, dec_b=dec_b, out_wT=out_wT, out_b=out_b)


def kernel(**inputs):
    x_enc = np.asarray(inputs["x_enc"], np.float32)
    seq_last = x_enc[:, -1:, :]                       # [B, 1, C]
    x = x_enc - seq_last                              # [B, L, C]

    weights = _prep_weights({k: np.asarray(v) for k, v in inputs.items()
                             if k not in ("x_enc", "x_mark_enc", "x_dec",
                                          "x_mark_dec")})

    core_ids = list(range(NCORES))
    in_maps = []
    for ci in core_ids:
        xs = x[ci * BS : (ci + 1) * BS]               # [BS, L, C]
        xT = np.ascontiguousarray(xs.transpose(1, 2, 0))  # [L, C, BS]
        m = dict(weights)
        m["xT"] = xT
        in_maps.append(m)

    nc = _get_nc()
    res = run_bass_kernel_spmd(nc, in_maps, core_ids)

    out = np.empty((B, T, C), np.float32)
    for ci in core_ids:
        p = res.results[ci]["preds"]                  # [T, C, BS]
        out[ci * BS : (ci + 1) * BS] = p.transpose(2, 0, 1)
    out += seq_last
    return out
